# revision 3
# baseline (speedup 1.0000x reference)
"""Trainium2 Bass kernel for nn_LinkPredictor (2-layer GCN + edge-dot decode), v2.

Key ideas vs v1:
  - dinv folding: tables hold u = dinv*h, so edge messages need NO per-edge
    norm -> S matrices are pure one-hot (iota == dstloc), built batched on DVE
    with a 0-stride broadcast tensor_tensor (16 tiles per instruction).
  - Quarter-sharded tables: each stage's node table is split into 4 quarter
    tables (src chunks == quarters), AllGathered independently as soon as the
    producing quarter of the shard is computed -> collectives overlap compute.
  - Decode groups ordered by max quarter needed -> decode overlaps L2 tail.
  - Self-loop: out[d] = dinv[d]*(segsum(u)[d] + u[d]) + b, u from SBUF.
"""
import contextlib
import math
import os
import numpy as np
import ml_dtypes

SERIAL_AG = False

import concourse.bass as bass
import concourse.tile as tile
from concourse import bacc, mybir
from concourse.bass_utils import run_bass_kernel_spmd
from concourse.tile_rust import add_dep_helper

F32 = mybir.dt.float32
BF16 = mybir.dt.bfloat16
I16 = mybir.dt.int16
BF = ml_dtypes.bfloat16


class Cfg:
    def __init__(self, N=100000, E=1600000, EL=100000, D=128, ncores=8,
                 nw=98, wb=4):
        self.N, self.E, self.EL, self.D, self.NC = N, E, EL, D, ncores
        self.NW = nw                      # windows (128 nodes each) per core
        self.SHARD = nw * 128             # nodes per core (padded)
        self.NP = self.SHARD * ncores     # padded node count
        assert self.NP >= N
        self.QW = [25, 25, 24, 24]        # windows per quarter
        assert sum(self.QW) == nw
        self.QS = np.concatenate([[0], np.cumsum(self.QW)])  # window starts
        self.NCH = 4                      # src chunks == quarters
        self.CHROWS = [ncores * q * 128 for q in self.QW]    # quarter tbl rows
        assert max(self.CHROWS) <= 32768
        self.WB = wb                      # windows per gather/aggregate batch
        self.NBATCH = math.ceil(nw / wb)


DEFAULT = Cfg()


def _wrap_idxs(idx):
    """[n] ints -> [128, n//16] int16 wrapped in 16 partitions, replicated 8x."""
    n = len(idx)
    assert n % 16 == 0
    w = np.asarray(idx, dtype=np.int16).reshape(n // 16, 16).T
    return np.ascontiguousarray(np.tile(w, (8, 1)))


def host_prep(cfg, x, edge_index, edge_label_index, W1, b1, W2, b2):
    c = cfg
    src = np.asarray(edge_index[0], dtype=np.int64)
    dst = np.asarray(edge_index[1], dtype=np.int64)
    deg = np.bincount(dst, minlength=c.N).astype(np.float64) + 1.0
    dinv = 1.0 / np.sqrt(deg)                      # [N]
    dinv_p = np.ones(c.NP, dtype=np.float64)
    dinv_p[:c.N] = dinv

    # node -> (core, window, lane) and position inside its quarter table
    def qpos(n):
        cc = n // c.SHARD
        r = n - cc * c.SHARD
        w = r // 128
        lane = r - w * 128
        q = np.searchsorted(c.QS, w, side="right") - 1
        return q, cc * (np.array(c.QW)[q] * 128) + (w - c.QS[q]) * 128 + lane

    q_src, pos_src = qpos(src)
    core_of = dst // c.SHARD
    w_of = (dst - core_of * c.SHARD) // 128

    key = (core_of * c.NW + w_of) * c.NCH + q_src
    order = np.argsort(key, kind="stable")
    ngroups = c.NC * c.NW * c.NCH
    counts = np.bincount(key[order], minlength=ngroups)
    B = int(128 * math.ceil(max(int(counts.max()), 1) / 128))
    starts = np.zeros(ngroups + 1, dtype=np.int64)
    np.cumsum(counts, out=starts[1:])

    TPG = B // 128
    TOT = c.NW * c.NCH * B
    TOT_TILES = TOT // 128

    idx_arr = np.zeros((c.NC, TOT), dtype=np.int64)
    dstloc_arr = np.full((c.NC, TOT), -1.0, dtype=np.float32)
    for core in range(c.NC):
        pos = 0
        for b in range(c.NBATCH):
            wlo, whi = b * c.WB, min((b + 1) * c.WB, c.NW)
            for ch in range(c.NCH):
                for w in range(wlo, whi):
                    g = (core * c.NW + w) * c.NCH + ch
                    eids = order[starts[g]:starts[g + 1]]
                    n = len(eids)
                    idx_arr[core, pos:pos + n] = pos_src[eids]
                    dstloc_arr[core, pos:pos + n] = (
                        dst[eids] - core * c.SHARD - w * 128)
                    pos += B
        assert pos == TOT

    # decode: label edge j -> core j // ELC; groups (qs, qd) sorted by need
    assert c.EL % c.NC == 0
    ELC = c.EL // c.NC
    ls = np.asarray(edge_label_index[0], dtype=np.int64)
    ld = np.asarray(edge_label_index[1], dtype=np.int64)
    qs_l, pos_l = qpos(ls)
    qd_l, pos_d = qpos(ld)
    grp_order = sorted(range(16), key=lambda g: (max(g // 4, g % 4), g))
    grp_rank = {g: i for i, g in enumerate(grp_order)}
    kd_raw = qs_l * 4 + qd_l
    kd = np.array([grp_rank[g] for g in kd_raw.tolist()], dtype=np.int64)
    NG_DEC = 16
    B_dec = 0
    for core in range(c.NC):
        cnt = np.bincount(kd[core * ELC:(core + 1) * ELC], minlength=NG_DEC)
        B_dec = max(B_dec, int(cnt.max()))
    B_dec = 128 * math.ceil(max(B_dec, 1) / 128)
    TOT_DEC = NG_DEC * B_dec
    idx_s = np.zeros((c.NC, TOT_DEC), dtype=np.int64)
    idx_d = np.zeros((c.NC, TOT_DEC), dtype=np.int64)
    slot2j = np.full((c.NC, TOT_DEC), -1, dtype=np.int64)
    for core in range(c.NC):
        jlo = core * ELC
        kk = kd[jlo:jlo + ELC]
        o = np.argsort(kk, kind="stable")
        cnt = np.bincount(kk, minlength=NG_DEC)
        st = np.zeros(NG_DEC + 1, dtype=np.int64)
        np.cumsum(cnt, out=st[1:])
        for g in range(NG_DEC):
            js = o[st[g]:st[g + 1]] + jlo
            n = len(js)
            pos = g * B_dec
            idx_s[core, pos:pos + n] = pos_l[js]
            idx_d[core, pos:pos + n] = pos_d[js]
            slot2j[core, pos:pos + n] = js

    dinv_f = dinv_p.astype(np.float32)
    xu = np.zeros((c.NP, c.D), dtype=np.float32)
    xu[:c.N] = np.asarray(x, dtype=np.float32)
    xu *= dinv_f[:, None]
    in_maps = []
    for core in range(c.NC):
        sl = slice(core * c.SHARD, (core + 1) * c.SHARD)
        in_maps.append({
            "xT": np.ascontiguousarray(xu[sl].T).astype(BF),
            "W1": np.asarray(W1, dtype=np.float32).astype(BF),
            "W2": np.asarray(W2, dtype=np.float32).astype(BF),
            "b1r": np.tile(np.asarray(b1, np.float32)[None, :], (128, 1)),
            "b2r": np.tile(np.asarray(b2, np.float32)[None, :], (128, 1)),
            "dinvc": np.ascontiguousarray(
                dinv_f[sl].reshape(c.NW, 128).T),
            "dinv2c": np.ascontiguousarray(
                (dinv_f[sl] ** 2).reshape(c.NW, 128).T),
            "dinq": (1.0 / dinv_f[sl]).reshape(1, c.SHARD).astype(BF),
            "b1q": np.asarray(b1, np.float32).reshape(1, c.D).astype(BF),
            "b2q": np.asarray(b2, np.float32).reshape(1, c.D).astype(BF),
            "gidx": _wrap_idxs(idx_arr[core]),
            "dstloc": np.ascontiguousarray(
                dstloc_arr[core].reshape(TOT_TILES, 128).T).astype(BF),
            "didx_s": _wrap_idxs(idx_s[core]),
            "didx_d": _wrap_idxs(idx_d[core]),
        })
    meta = dict(B=B, TPG=TPG, TOT=TOT, TOT_TILES=TOT_TILES,
                B_dec=B_dec, TOT_DEC=TOT_DEC, slot2j=slot2j,
                grp_order=grp_order)
    return in_maps, meta


def build_program(cfg, meta, num_cores=None):
    c = cfg
    NCores = num_cores or c.NC
    B, TPG, TOT, TOT_TILES = meta["B"], meta["TPG"], meta["TOT"], meta["TOT_TILES"]
    B_dec, TOT_DEC = meta["B_dec"], meta["TOT_DEC"]
    grp_order = meta["grp_order"]
    D = c.D
    NQ = 4

    nc = bacc.Bacc("TRN2", target_bir_lowering=False, debug=False,
                   num_devices=NCores, num_swdge_queues=NQ)

    xT_in = nc.dram_tensor("xT", [D, c.SHARD], BF16, kind="ExternalInput")
    W1_in = nc.dram_tensor("W1", [D, D], BF16, kind="ExternalInput")
    W2_in = nc.dram_tensor("W2", [D, D], BF16, kind="ExternalInput")
    b1_in = nc.dram_tensor("b1r", [128, D], F32, kind="ExternalInput")
    b2_in = nc.dram_tensor("b2r", [128, D], F32, kind="ExternalInput")
    dinv_in = nc.dram_tensor("dinvc", [128, c.NW], F32, kind="ExternalInput")
    dinv2_in = nc.dram_tensor("dinv2c", [128, c.NW], F32, kind="ExternalInput")
    dinq_in = nc.dram_tensor("dinq", [1, c.SHARD], BF16, kind="ExternalInput")
    b1q_in = nc.dram_tensor("b1q", [1, c.D], BF16, kind="ExternalInput")
    b2q_in = nc.dram_tensor("b2q", [1, c.D], BF16, kind="ExternalInput")
    gidx_in = nc.dram_tensor("gidx", [128, TOT // 16], I16, kind="ExternalInput")
    dstloc_in = nc.dram_tensor("dstloc", [128, TOT_TILES], BF16, kind="ExternalInput")
    didx_s_in = nc.dram_tensor("didx_s", [128, TOT_DEC // 16], I16, kind="ExternalInput")
    didx_d_in = nc.dram_tensor("didx_d", [128, TOT_DEC // 16], I16, kind="ExternalInput")
    dots_out = nc.dram_tensor("dots", [128, TOT_DEC // 128], F32, kind="ExternalOutput")

    # quarter shard/table DRAM tensors for the 3 stages (u1, v1, z2)
    shards = [[nc.dram_tensor(f"sh{s}_{q}", [c.QW[q] * 128, D], BF16)
               for q in range(4)] for s in range(3)]
    tables = [[nc.dram_tensor(f"tb{s}_{q}", [NCores * c.QW[q] * 128, D], BF16)
               for q in range(4)] for s in range(3)]
    boot_in = nc.dram_tensor("bootin", [16, 8], F32)
    boot_out = nc.dram_tensor("bootout", [16 * NCores, 8], F32)

    iotaw_dram = nc.inline_tensor(
        np.tile(np.arange(128, dtype=np.float32), (128, 16)).astype(BF), "iotaw_c")
    ident_dram = nc.inline_tensor(np.eye(128, dtype=np.float32).astype(BF), "ident_c")

    serial_cc_sem = nc.alloc_semaphore("serial_cc") if SERIAL_AG else None
    core_ids = list(range(NCores))
    shard_dmas = {(s, q): [] for s in range(3) for q in range(4)}

    # gpsimd program-order chain: gathers, collective triggers, waits
    gst = {"count": 0, "prev": None}
    ag_insts = {}

    def gp(inst):
        if gst["prev"] is not None:
            add_dep_helper(inst.ins, gst["prev"].ins, sync=False,
                           reason="gpsimd order")
        gst["prev"] = inst
        return inst

    def emit_gather(out_ap, in_ap, idx_ap, n_idx, dep=None):
        q = gst["count"] % NQ
        gst["count"] += 1
        inst = gp(nc.gpsimd.dma_gather(out_ap, in_ap, idx_ap, n_idx, n_idx, D,
                                       queue_num=q, single_packet=False))
        if dep is not None:
            add_dep_helper(inst.ins, dep.ins, sync=True, reason="table ready")
        return inst

    def trigger_ag(stage, q):
        nop = gp(nc.gpsimd.nop())
        for dma in shard_dmas[(stage, q)]:
            add_dep_helper(nop.ins, dma.ins, sync=True, reason="shard ready")
        inst = nc.gpsimd.collective_compute(
            "AllGather", mybir.AluOpType.bypass,
            replica_groups=[core_ids],
            ins=[shards[stage][q][:]], outs=[tables[stage][q][:]])
        gp(inst)
        ag_insts[(stage, q)] = inst

    serial_cc = {"n": 0}

    def serial_ag_stage(tc, stage):
        """Baseline-style serial AllGather of all 4 quarters of a stage."""
        tc.strict_bb_all_engine_barrier()
        with tc.tile_critical():
            for q in range(4):
                nc.gpsimd.collective_compute(
                    "AllGather", mybir.AluOpType.bypass,
                    replica_groups=[core_ids],
                    ins=[shards[stage][q][:]], outs=[tables[stage][q][:]],
                ).then_inc(serial_cc_sem)
                serial_cc["n"] += 1
            nc.gpsimd.wait_ge(serial_cc_sem, serial_cc["n"])
        tc.strict_bb_all_engine_barrier()

    with tile.TileContext(nc) as tc:
        with contextlib.ExitStack() as es:
            # boot collective: absorb core launch skew early
            if not SERIAL_AG:
                boot = nc.gpsimd.collective_compute(
                    "AllGather", mybir.AluOpType.bypass,
                    replica_groups=[core_ids],
                    ins=[boot_in[:]], outs=[boot_out[:]])
                gp(boot)

            const = es.enter_context(tc.tile_pool(name="const", bufs=1))
            meta_p = es.enter_context(tc.tile_pool(name="meta", bufs=1))

            w1_sb = const.tile([D, D], BF16); nc.sync.dma_start(w1_sb[:], W1_in[:])
            w2_sb = const.tile([D, D], BF16); nc.sync.dma_start(w2_sb[:], W2_in[:])
            b1_sb = const.tile([128, D], F32); nc.sync.dma_start(b1_sb[:], b1_in[:])
            b2_sb = const.tile([128, D], F32); nc.sync.dma_start(b2_sb[:], b2_in[:])
            dinv_sb = const.tile([128, c.NW], F32)
            nc.sync.dma_start(dinv_sb[:], dinv_in[:])
            dinv2_sb = const.tile([128, c.NW], F32)
            nc.sync.dma_start(dinv2_sb[:], dinv2_in[:])
            dinq_sb = const.tile([1, c.SHARD], BF16)
            nc.sync.dma_start(dinq_sb[:], dinq_in[:])
            b1q_sb = const.tile([1, c.D], BF16)
            nc.sync.dma_start(b1q_sb[:], b1q_in[:])
            b2q_sb = const.tile([1, c.D], BF16)
            nc.sync.dma_start(b2q_sb[:], b2q_in[:])
            ones1_sb = const.tile([1, 128], BF16)
            nc.vector.memset(ones1_sb[:], 1.0)
            iotaw_sb = const.tile([128, 16, 128], BF16)
            nc.sync.dma_start(
                iotaw_sb[:].rearrange("p a b -> p (a b)"), iotaw_dram[:])
            ident_sb = const.tile([128, 128], BF16)
            nc.sync.dma_start(ident_sb[:], ident_dram[:])
            gidx_sb = meta_p.tile([128, TOT // 16], I16)
            nc.sync.dma_start(gidx_sb[:], gidx_in[:])
            dstloc_sb = meta_p.tile([128, TOT_TILES], BF16)
            nc.sync.dma_start(dstloc_sb[:], dstloc_in[:])

            def q_of_w(w):
                return int(np.searchsorted(c.QS, w, side="right") - 1)

            pending = []   # [(stage, q)] AG triggers not yet emitted

            def flush(stage_lt_w=None, all_=False):
                if SERIAL_AG:
                    return
                while pending:
                    s_, q_ = pending[0]
                    if all_ or (stage_lt_w is not None
                                and c.QS[q_ + 1] + c.WB <= stage_lt_w):
                        trigger_ag(s_, q_)
                        pending.pop(0)
                    else:
                        break

            def shard_write(stage, w, t):
                q = q_of_w(w)
                row = (w - int(c.QS[q])) * 128
                dma = nc.sync.dma_start(
                    shards[stage][q][row:row + 128, :], t[:])
                if not SERIAL_AG:
                    shard_dmas[(stage, q)].append(dma)
                    if w == int(c.QS[q + 1]) - 1:
                        pending.append((stage, q))

            def layer(lid, stage_in, h_tiles, bias_sb, out_pool,
                      make_next):
                out_tiles = []
                with tc.tile_pool(name=f"M{lid}", bufs=2) as Mp, \
                     tc.tile_pool(name=f"S{lid}", bufs=7) as Sp, \
                     tc.tile_pool(name=f"ag{lid}", bufs=4, space="PSUM") as agp, \
                     tc.tile_pool(name=f"tp{lid}", bufs=2, space="PSUM") as tpp, \
                     tc.tile_pool(name=f"ep{lid}", bufs=3) as epp:
                    for b in range(c.NBATCH):
                        wlo = b * c.WB
                        whi = min(wlo + c.WB, c.NW)
                        nwb = whi - wlo
                        flush(stage_lt_w=wlo)
                        cols_per_ch = nwb * TPG
                        ncols = c.NCH * cols_per_ch
                        Mt = Mp.tile([128, ncols, D], BF16, tag="M")
                        slot_base = wlo * c.NCH * B
                        for ch in range(c.NCH):
                            n_idx = nwb * B
                            off16 = (slot_base + ch * n_idx) // 16
                            emit_gather(
                                Mt[:, ch * cols_per_ch:(ch + 1) * cols_per_ch, :],
                                tables[stage_in][ch][:],
                                gidx_sb[:, off16:off16 + n_idx // 16],
                                n_idx,
                                dep=ag_insts[(stage_in, ch)]
                                if (b == 0 and not SERIAL_AG) else None)
                        tile_base = slot_base // 128
                        # batched one-hot S builds (16 tiles per DVE op)
                        s_tiles = []
                        for j in range(0, ncols, 16):
                            jn = min(16, ncols - j)
                            Sw = Sp.tile([128, 16, 128], BF16, tag="S")
                            bc = dstloc_sb[:, tile_base + j:tile_base + j + jn]\
                                .unsqueeze(2).broadcast_to([128, jn, 128])
                            nc.vector.tensor_tensor(
                                Sw[:, :jn, :], iotaw_sb[:, :jn, :], bc,
                                op=mybir.AluOpType.is_equal)
                            s_tiles.append(Sw)
                        for wi in range(nwb):
                            w = wlo + wi
                            ps = agp.tile([128, D], F32, tag="agg")
                            nmm = c.NCH * TPG
                            k = 0
                            for ch in range(c.NCH):
                                for t in range(TPG):
                                    mcol = (ch * nwb + wi) * TPG + t
                                    S = s_tiles[mcol // 16][:, mcol % 16, :]
                                    nc.tensor.matmul(
                                        ps[:], lhsT=S, rhs=Mt[:, mcol, :],
                                        start=(k == 0), stop=(k == nmm - 1))
                                    k += 1
                            s1 = epp.tile([128, D], F32, tag="s1")
                            nc.vector.tensor_tensor(
                                s1[:], ps[:], h_tiles[w][:],
                                op=mybir.AluOpType.add)
                            if make_next:
                                # v1 = relu(dinv*(dinv*s1 + b1)) = dinv*z1
                                c1 = epp.tile([128, D], F32, tag="c1")
                                nc.vector.tensor_scalar(
                                    c1[:], s1[:], dinv_sb[:, w:w + 1], None,
                                    mybir.AluOpType.mult)
                                c2 = epp.tile([128, D], F32, tag="c2")
                                nc.vector.tensor_tensor(
                                    c2[:], c1[:], bias_sb[:],
                                    op=mybir.AluOpType.add)
                                ht = out_pool.tile([128, D], BF16, tag="nxt")
                                nc.scalar.activation(
                                    ht[:], c2[:],
                                    mybir.ActivationFunctionType.Relu,
                                    scale=dinv_sb[:, w:w + 1])
                            else:
                                # z2 = relu((dinv*s1) @ W2 + b2)
                                y1 = epp.tile([128, D], BF16, tag="y1")
                                nc.vector.tensor_scalar(
                                    y1[:], s1[:], dinv_sb[:, w:w + 1], None,
                                    mybir.AluOpType.mult)
                                yt_ps = tpp.tile([128, D], BF16, tag="yt")
                                nc.tensor.transpose(yt_ps[:], y1[:], ident_sb[:])
                                yT = epp.tile([128, D], BF16, tag="yT")
                                nc.vector.tensor_copy(yT[:], yt_ps[:])
                                h2ps = tpp.tile([128, D], F32, tag="h2")
                                nc.tensor.matmul(h2ps[:], lhsT=yT[:], rhs=w2_sb[:],
                                                 start=True, stop=True)
                                c2 = epp.tile([128, D], F32, tag="c2b")
                                nc.vector.tensor_tensor(
                                    c2[:], h2ps[:], bias_sb[:],
                                    op=mybir.AluOpType.add)
                                ht = out_pool.tile([128, D], BF16, tag="nxt")
                                nc.scalar.activation(
                                    ht[:], c2[:],
                                    mybir.ActivationFunctionType.Relu)
                            shard_write(lid, w, ht)
                            out_tiles.append(ht)
                return out_tiles

            # P0: u1 = (dinv*x) @ W1 per window
            h1_tiles = []
            with tc.tile_pool(name="hsb2", bufs=c.NW) as hsb2:
                with tc.tile_pool(name="hsb1", bufs=c.NW) as hsb1:
                    with tc.tile_pool(name="p0", bufs=3) as p0, \
                         tc.tile_pool(name="p0ps", bufs=2, space="PSUM") as p0ps:
                        for w in range(c.NW):
                            xt = p0.tile([D, 128], BF16)
                            nc.sync.dma_start(
                                xt[:], xT_in[:, w * 128:(w + 1) * 128])
                            ps = p0ps.tile([128, D], F32, tag="ps")
                            nc.tensor.matmul(ps[:], lhsT=xt[:], rhs=w1_sb[:],
                                             start=True, stop=True)
                            h1t = hsb1.tile([128, D], BF16, tag="h1t")
                            nc.vector.tensor_copy(h1t[:], ps[:])
                            shard_write(0, w, h1t)
                            h1_tiles.append(h1t)
                    flush(all_=True)   # AG1 quarters
                    h2_tiles = layer(1, 0, h1_tiles, b1_sb, hsb2,
                                     make_next=True)
                    del h1_tiles[:]
                if SERIAL_AG:
                    serial_ag_stage(tc, 1)
                flush(all_=True)   # any remaining AG2 quarters
                with tc.tile_pool(name="zsink", bufs=4) as zsink:
                    layer(2, 1, h2_tiles, b2_sb, zsink,
                          make_next=False)
                if SERIAL_AG:
                    serial_ag_stage(tc, 2)

                # decode, group-by-group as z quarters arrive
                with tc.tile_pool(name="didx", bufs=1) as didxp, \
                     tc.tile_pool(name="dM", bufs=6) as dMp, \
                     tc.tile_pool(name="dw", bufs=4) as dwp, \
                     tc.tile_pool(name="dout", bufs=1) as doutp:
                    ds_sb = didxp.tile([128, TOT_DEC // 16], I16)
                    nc.sync.dma_start(ds_sb[:], didx_s_in[:])
                    dd_sb = didxp.tile([128, TOT_DEC // 16], I16)
                    nc.sync.dma_start(dd_sb[:], didx_d_in[:])
                    res = doutp.tile([128, TOT_DEC // 128], F32)
                    dec_seen = set()
                    for gi, g in enumerate(grp_order):
                        qs, qd = g // 4, g % 4
                        mq = max(qs, qd)
                        # ensure AG3 quarters <= mq are triggered
                        while pending and pending[0][1] <= mq:
                            trigger_ag(*pending.pop(0))
                        off16 = gi * B_dec // 16
                        coff = gi * B_dec // 128
                        nccol = B_dec // 128
                        Ms = dMp.tile([128, nccol, D], BF16, tag="Ms")
                        Md = dMp.tile([128, nccol, D], BF16, tag="Md")
                        emit_gather(Ms[:], tables[2][qs][:],
                                    ds_sb[:, off16:off16 + B_dec // 16],
                                    B_dec,
                                    dep=None if (qs in dec_seen or SERIAL_AG)
                                    else ag_insts[(2, qs)])
                        dec_seen.add(qs)
                        emit_gather(Md[:], tables[2][qd][:],
                                    dd_sb[:, off16:off16 + B_dec // 16],
                                    B_dec,
                                    dep=None if (qd in dec_seen or SERIAL_AG)
                                    else ag_insts[(2, qd)])
                        dec_seen.add(qd)
                        for col in range(nccol):
                            mm = dwp.tile([128, D], F32, tag="mm")
                            if False:
                                nc.vector.tensor_tensor_reduce(
                                    mm[:], Ms[:, col, :], Md[:, col, :],
                                    1.0, 0.0,
                                    mybir.AluOpType.mult, mybir.AluOpType.add,
                                    res[:, coff + col:coff + col + 1])
                            else:
                                nc.vector.tensor_tensor(
                                    mm[:], Ms[:, col, :], Md[:, col, :],
                                    op=mybir.AluOpType.mult)
                                nc.vector.reduce_sum(
                                    res[:, coff + col:coff + col + 1], mm[:],
                                    axis=mybir.AxisListType.X)
                    nc.sync.dma_start(dots_out[:], res[:])

    nc.compile()
    return nc


def assemble_output(cfg, meta, results):
    c = cfg
    slot2j = meta["slot2j"]
    out = np.zeros(c.EL, dtype=np.float32)
    for core in range(len(results)):
        d = np.asarray(results[core]["dots"], dtype=np.float32)
        flat = d.T.reshape(-1)             # slot i -> d[i%128, i//128]
        s2j = slot2j[core]
        valid = s2j >= 0
        out[s2j[valid]] = flat[valid]
    return out


def run_pipeline(x, edge_index, edge_label_index, W1, b1, W2, b2,
                 cfg=None, trace=False, tmpdir=None):
    cfg = cfg or DEFAULT
    in_maps, meta = host_prep(cfg, x, edge_index, edge_label_index,
                              W1, b1, W2, b2)
    nc = build_program(cfg, meta)
    res = run_bass_kernel_spmd(nc, in_maps, list(range(cfg.NC)),
                               trace=trace, tmpdir=tmpdir)
    return assemble_output(cfg, meta, res.results), res


def kernel(x, edge_index, edge_label_index, W1, b1, W2, b2):
    out, _ = run_pipeline(x, edge_index, edge_label_index, W1, b1, W2, b2)
    return out
